# revision 49
# baseline (speedup 1.0000x reference)
"""Trainium2 Bass kernel for nn_DENIS_JBF (Koopman Jordan-block forecast).

v3 strategy (pure data parallel, 8 cores, Bc = 512 per core), all-bf16:
  - BN stats: host-packed partition-major Gram input (one DMA), bf16 PE
    Gram matmuls, AllGather (cheaper than AllReduce) + 3-step local
    reduce; BN folded into L1 weights on device.
  - One-pass leaky-relu evictions: ACT Prelu(alpha) from PSUM with L2
    bias fused (or skipped entirely when the biases are all zero, per
    input_specs); a tunable subset of tiles goes to DVE (copy+STT) to
    balance engines.
  - All-bf16: PE 1 col/cycle, DVE 2x_1p fast mode, half-size outputs.
  - Rotation in closed form, [128b, t, k] tiles with in-op broadcasts;
    Sin/Exp batched to avoid ACT table thrash; ec/es on GPSIMD (Pool).
  - yenc dims permuted (even/odd split) via host-permuted enc_W3 so
    mu/om and y0 pairs are contiguous after one blocked DMA-transpose.
  - x_pred: o0/o1 blocked-DMA-transposed, block-diagonal Cw matmuls.
  - DMA queues by role: SP = loads (deep xst prefetch) + stats + xp;
    GPSIMD/SWDGE = bulk stores (yencP, yl); ACT = dma transposes.
"""

import os
import sys

import numpy as np
import ml_dtypes

for _p in ("/opt/trn_rl_repo", "/root/.axon_site/_ro/trn_rl_repo"):
    if os.path.isdir(_p) and _p not in sys.path:
        sys.path.insert(0, _p)

import concourse.bass as bass
from concourse import bacc
import concourse.mybir as mybir
import concourse.tile as tile
from concourse import bass_utils
import concourse.dve_ops as dve_ops
from concourse.dve_ops import DveOp
from concourse.dve_spec import Spec, Src0, C0, maxx, lower
from concourse.dve_uop import DveOpSpec


def _register_lrelu():
    """Single-pass leaky-relu as a custom DVE op: max(Src0*C0, Src0)."""
    if "LRELU_ANT" in dve_ops._SUB_OPCODE_FOR_NAME:
        return next(op for op in dve_ops.OPS if op.name == "LRELU_ANT")
    spec = Spec(
        body=maxx(Src0 * C0, Src0),
        reference=lambda in0, in1, s0, s1, imm2: np.maximum(
            in0.astype(np.float32) * s0, in0.astype(np.float32)),
    )
    row = dve_ops._CUSTOM_DVE_ROW_BASE + len(dve_ops.OPS)
    assert row < 0x20
    dve_ops._SUB_OPCODE_FOR_NAME["LRELU_ANT"] = row
    shas = {}
    for ver in ("v3", "v4"):
        uops = lower(spec, ver=ver)
        shas[ver] = DveOpSpec(name="LRELU_ANT", opcode=row, uops=uops,
                              rd1_en=False).sha(ver)
    op = DveOp("LRELU_ANT", spec, subdim=False, uops_sha=shas)
    dve_ops.OPS.append(op)
    dve_ops.CUSTOM_DVE_SPECS["LRELU_ANT"] = spec
    return op


LRELU_OP = _register_lrelu()

F32 = mybir.dt.float32
BF16 = mybir.dt.bfloat16
AF = mybir.ActivationFunctionType
OP = mybir.AluOpType
AX = mybir.AxisListType

NCORES = 8
B, T, DIM, LDIM, NAUX = 4096, 64, 16, 64, 32
H, AH = 256, 128
DT = 0.01
EPS = 1e-5
BC = B // NCORES            # 512
COLS = BC * T               # 32768, col = t*BC + b
D1 = DIM + 1                # 17: +ones row (L1 bias in matmul)
PI = float(np.pi)
PF = 8                      # xst prefetch depth

# --- engine-assignment tunables -----------------------------------------
ENC_L1_DVE = frozenset(j for j in range(64) if j % 2 == 1)
ENC_L2_DVE = frozenset(j for j in range(64) if j % 4 == 1)
AUX_L1_DVE = frozenset(range(16))


def build(zb=True):
    """zb: biases (enc_b2 / aux_b2) are all-zero -> single-op L2 evictions."""
    nc = bacc.Bacc(None)

    # ---------------- DRAM I/O ----------------
    xsT_d = nc.dram_tensor("xsT", [D1, T, BC], BF16, kind="ExternalInput")
    xsNg_d = nc.dram_tensor("xsNg", [128, 256, 18], BF16, kind="ExternalInput")
    xsN0_d = nc.dram_tensor("xsN0", [128, 4, 18], BF16, kind="ExternalInput")
    w1e_d = nc.dram_tensor("w1e", [DIM, H], F32, kind="ExternalInput")
    b1r_d = nc.dram_tensor("b1r", [1, H], F32, kind="ExternalInput")
    w2_d = nc.dram_tensor("w2", [128, 4, 128], BF16, kind="ExternalInput")
    b2c_d = nc.dram_tensor("b2c", [128, 2], F32, kind="ExternalInput")
    w3_d = nc.dram_tensor("w3", [128, 2, LDIM], BF16, kind="ExternalInput")
    w1a_d = nc.dram_tensor("w1a", [DIM, NAUX * AH], F32, kind="ExternalInput")
    b1ar_d = nc.dram_tensor("b1ar", [1, NAUX * AH], BF16, kind="ExternalInput")
    w2a_d = nc.dram_tensor("w2a", [AH, NAUX, AH], BF16, kind="ExternalInput")
    b2ac_d = nc.dram_tensor("b2ac", [128, NAUX], F32, kind="ExternalInput")
    w3a_d = nc.dram_tensor("w3a", [AH, NAUX, LDIM], BF16, kind="ExternalInput")
    cwe_d = nc.dram_tensor("cwe", [128, 64], BF16, kind="ExternalInput")
    cwo_d = nc.dram_tensor("cwo", [128, 64], BF16, kind="ExternalInput")
    tvk_d = nc.dram_tensor("tvk", [128, T, 32], BF16, kind="ExternalInput")
    id16_d = nc.dram_tensor("id16", [DIM, DIM], F32, kind="ExternalInput")
    onesr_d = nc.dram_tensor("onesr", [1, BC], BF16, kind="ExternalInput")

    yencP_o = nc.dram_tensor("yencP", [32, 128, BC], BF16, kind="ExternalOutput")
    yl0_o = nc.dram_tensor("yl0", [4, 128, T, 32], BF16, kind="ExternalOutput")
    yl1_o = nc.dram_tensor("yl1", [4, 128, T, 32], BF16, kind="ExternalOutput")
    xp_o = nc.dram_tensor("xp", [4, 2, 128, 512], BF16, kind="ExternalOutput")

    stat_in = nc.dram_tensor("stat_in", [2, 18, 18], F32)
    stat_out = nc.dram_tensor("stat_out", [NCORES, 2, 18, 18], F32,
                              addr_space="Shared")

    with tile.TileContext(nc) as tc:
        with tc.tile_pool(name="consts", bufs=1) as cp, \
             tc.tile_pool(name="psum", bufs=1, space="PSUM") as pp, \
             tc.tile_pool(name="stream", bufs=2) as sp, \
             tc.tile_pool(name="smalls", bufs=1) as smp:
            bp = tc.alloc_tile_pool(name="boot", bufs=1)

            # ------------- weights / consts (SP loads, overlap stats) ----
            def ld(shape, dt, dram, name):
                t = cp.tile(shape, dt, tag=name)
                nc.sync.dma_start(out=t, in_=dram)
                return t

            # stats inputs first: the Gram + AllGather is the serial prologue
            xg = bp.tile([128, 256, 18], BF16, tag="xg", bufs=1)
            nc.sync.dma_start(out=xg, in_=xsNg_d[:, :, :])
            x0g = bp.tile([128, 4, 18], BF16, tag="x0g", bufs=1)
            nc.sync.dma_start(out=x0g, in_=xsN0_d[:, :, :])
            w1e_sb = ld([DIM, H], F32, w1e_d[:, :], "w1e")
            b1r_sb = ld([1, H], F32, b1r_d[:, :], "b1r")
            w2_sb = ld([128, 4, 128], BF16, w2_d[:, :, :], "w2")
            b2c_sb = ld([128, 2], F32, b2c_d[:, :], "b2c")
            w3_sb = ld([128, 2, LDIM], BF16, w3_d[:, :, :], "w3")
            w2a_sb = ld([AH, NAUX, AH], BF16, w2a_d[:, :, :], "w2a")
            b2ac_sb = ld([128, NAUX], F32, b2ac_d[:, :], "b2ac")
            w3a_sb = ld([AH, NAUX, LDIM], BF16, w3a_d[:, :, :], "w3a")
            cwe_sb = ld([128, 64], BF16, cwe_d[:, :], "cwe")
            cwo_sb = ld([128, 64], BF16, cwo_d[:, :], "cwo")
            tvk_sb = ld([128, T, 32], BF16, tvk_d[:, :, :], "tvk")
            id16_sb = ld([DIM, DIM], F32, id16_d[:, :], "id16")
            w1a_sb = bp.tile([DIM, NAUX * AH], F32, tag="w1atmp", bufs=1)
            nc.sync.dma_start(out=w1a_sb, in_=w1a_d[:, :])
            x0T = bp.tile([DIM, BC], BF16, tag="x0T", bufs=1)
            nc.sync.dma_start(out=x0T, in_=xsT_d[0:DIM, 0, :])

            hpib = cp.tile([128, 1], F32)
            nc.vector.memset(hpib, PI / 2.0)
            epsb = cp.tile([128, 1], F32)
            nc.vector.memset(epsb, EPS)

            # xst stream with deep prefetch (pure loads on SP queue)
            def load_xst(jb):
                xst = sp.tile([D1, 2, 512], BF16, tag="xst", bufs=PF)
                nc.sync.dma_start(
                    out=xst, in_=xsT_d[:, jb * 2:(jb + 1) * 2, :])
                return xst

            xst_tiles = {}
            for jb in range(min(PF, 32)):
                xst_tiles[jb] = load_xst(jb)

            # ------------- phase A: Gram stats + AllGather -------------
            pg0 = pp.tile([18, 18], F32, tag="pC", bufs=2)
            for g in range(4):
                nc.tensor.matmul(pg0[:, :], x0g[:, g, :], x0g[:, g, :],
                                 start=(g == 0), stop=(g == 3))
            pg = pp.tile([18, 18], F32, tag="pA0", bufs=2)
            for g in range(256):
                nc.tensor.matmul(pg[:, :], xg[:, g, :], xg[:, g, :],
                                 start=(g == 0), stop=(g == 255))
            gA = smp.tile([18, 18], F32)
            nc.vector.tensor_copy(gA, pg[:, :])
            gB = smp.tile([18, 18], F32)
            nc.vector.tensor_copy(gB, pg0[:, :])
            nc.gpsimd.dma_start(out=stat_in[0, :, :], in_=gA[:])
            nc.gpsimd.dma_start(out=stat_in[1, :, :], in_=gB[:])
            nc.gpsimd.collective_compute(
                "AllGather", OP.bypass, replica_groups=[list(range(NCORES))],
                ins=[stat_in[:, :, :]], outs=[stat_out[:, :, :, :]])
            g8 = smp.tile([18, NCORES, 2, 18], F32)
            nc.gpsimd.dma_start(out=g8,
                                in_=stat_out[:, :, :, :].transpose([2, 0, 1, 3]))
            r4 = smp.tile([18, 4, 2, 18], F32)
            nc.vector.tensor_add(r4, g8[:, 0:4, :, :], g8[:, 4:8, :, :])
            r2 = smp.tile([18, 2, 2, 18], F32)
            nc.vector.tensor_add(r2, r4[:, 0:2, :, :], r4[:, 2:4, :, :])
            stats = smp.tile([18, 2, 18], F32)
            nc.vector.tensor_add(stats, r2[:, 0, :, :], r2[:, 1, :, :])

            # ------------- phase F: fold BN into L1 weights -------------
            def fold(set_idx, n):
                g16 = stats[0:DIM, set_idx, 0:DIM]
                scol = stats[0:DIM, set_idx, DIM:DIM + 1]
                m = smp.tile([DIM, 1], F32, tag=f"m{set_idx}")
                nc.vector.tensor_scalar(m, scol, 1.0 / n, None, OP.mult)
                gi = smp.tile([DIM, DIM], F32, tag=f"gi{set_idx}")
                nc.vector.tensor_mul(gi, g16, id16_sb[:])
                qd = smp.tile([DIM, 1], F32, tag=f"qd{set_idx}")
                nc.vector.reduce_sum(qd, gi, axis=AX.X)
                m2 = smp.tile([DIM, 1], F32, tag=f"m2{set_idx}")
                nc.vector.tensor_mul(m2, m, m)
                v = smp.tile([DIM, 1], F32, tag=f"v{set_idx}")
                nc.vector.scalar_tensor_tensor(v, qd, 1.0 / n, m2,
                                               OP.mult, OP.subtract)
                # rs = (v+eps)^-1/2 via Ln+Exp (one shared act table set)
                lnv = smp.tile([DIM, 1], F32, tag=f"ln{set_idx}")
                nc.scalar.activation(lnv, v, AF.Ln, bias=epsb[0:DIM, :])
                rs = smp.tile([DIM, 1], F32, tag=f"rs{set_idx}")
                nc.scalar.activation(rs, lnv, AF.Exp, scale=-0.5)
                return m, rs

            m_all, rs_all = fold(0, float(B * T))
            m_0, rs_0 = fold(1, float(B))

            w1es = cp.tile([D1, H], BF16)
            nc.vector.tensor_scalar(w1es[0:DIM, :], w1e_sb[:], rs_all[:],
                                    None, OP.mult)
            mneg = smp.tile([DIM, 1], BF16)
            nc.vector.tensor_scalar(mneg, m_all[:], -1.0, None, OP.mult)
            pbc = pp.tile([1, H], F32, tag="pC", bufs=2)
            nc.tensor.matmul(pbc[:, :], mneg[:], w1es[0:DIM, :],
                             start=True, stop=True)
            badd = smp.tile([1, H], BF16)
            nc.vector.tensor_add(badd, pbc[:, :], b1r_sb[:])
            nc.sync.dma_start(out=w1es[DIM:D1, :], in_=badd[:])

            w1as = cp.tile([D1, NAUX * AH], BF16)
            nc.vector.tensor_scalar(w1as[0:DIM, 0:2048], w1a_sb[:, 0:2048],
                                    rs_0[:], None, OP.mult)
            nc.scalar.activation(w1as[0:DIM, 2048:4096],
                                 w1a_sb[:, 2048:4096], AF.Copy,
                                 scale=rs_0[:])
            nc.sync.dma_start(out=w1as[DIM:D1, :], in_=b1ar_d[:, :])

            m0n = smp.tile([DIM, 1], F32)
            nc.vector.tensor_scalar(m0n, m_0[:], -1.0, None, OP.mult)
            x0c = cp.tile([D1, BC], BF16)
            nc.vector.tensor_scalar(x0c[0:DIM, :], x0T[:], m0n[:],
                                    None, OP.add)
            nc.sync.dma_start(out=x0c[DIM:D1, :], in_=onesr_d[:, :])
            bp.release()
            rp = tc.alloc_tile_pool(name="rot", bufs=1)

            # ------------- phase X: aux nets -> pmw [64, 512] -------------
            pmw = pp.tile([LDIM, BC], F32, tag="pC", bufs=2)
            for kp in range(NAUX // 2):
                h1a = sp.tile([128, 2, BC], BF16, tag="h1a", bufs=2)
                pa1 = [pp.tile([128, BC], F32, tag=f"pA{s}", bufs=2,
                               name=f"pa1_{s}") for s in range(2)]
                for s in range(2):
                    k = kp * 2 + s
                    nc.tensor.matmul(pa1[s][:, :],
                                     w1as[:, k * AH:(k + 1) * AH],
                                     x0c[:], start=True, stop=True)
                for s in range(2):
                    if kp in AUX_L1_DVE:
                        nc.vector._custom_dve(LRELU_OP, out=h1a[:, s, :],
                                              in0=pa1[s][:, :], s0=0.01)
                    else:
                        nc.scalar.activation(h1a[:, s, :], pa1[s][:, :],
                                             AF.Prelu, alpha=0.01)
                h2a = sp.tile([128, 2, BC], BF16, tag="h2a", bufs=2)
                for s in range(2):
                    k = kp * 2 + s
                    pa2 = pp.tile([128, BC], F32, tag=f"pB{s}", bufs=1)
                    nc.tensor.matmul(pa2[:, :], w2a_sb[:, k, :],
                                     h1a[:, s, :], start=True, stop=True)
                    if zb:
                        nc.scalar.activation(h2a[:, s, :], pa2[:, :],
                                             AF.Prelu, alpha=0.01)
                    else:
                        nc.scalar.activation(h2a[:, s, :], pa2[:, :],
                                             AF.Prelu,
                                             bias=b2ac_sb[:, k:k + 1],
                                             alpha=0.01)
                for s in range(2):
                    k = kp * 2 + s
                    nc.tensor.matmul(pmw[:, :], w3a_sb[:, k, :],
                                     h2a[:, s, :],
                                     start=(k == 0), stop=(k == NAUX - 1))

            # muom -> SBUF bf16, blocked transpose to b-major
            muomS = cp.tile([LDIM, BC], BF16)
            nc.vector.tensor_copy(muomS, pmw[:, :])
            mT = cp.tile([128, 4, LDIM], BF16)
            nc.sync.dma_start_transpose(mT, muomS[:])
            # mT[:, c, 0:32] = mu, mT[:, c, 32:64] = om

            # ------------- phase E: encoder -------------
            y0S = cp.tile([LDIM, BC], BF16)
            yT = cp.tile([128, 4, LDIM], BF16)

            def encoder_jb(jb):
                if jb + PF < 32:
                    xst_tiles[jb + PF] = load_xst(jb + PF)
                xst = xst_tiles.pop(jb)
                p3 = pp.tile([128, 512], F32, tag="pC", bufs=2)
                for jj in range(2):
                    j = jb * 2 + jj
                    rhs = xst[:, jj, :]
                    h1 = sp.tile([128, 2, 512], BF16, tag="h1")
                    p1 = [pp.tile([128, 512], F32, tag=f"pA{mo}", bufs=2,
                                  name=f"p1_{mo}") for mo in range(2)]
                    for mo in range(2):
                        nc.tensor.matmul(p1[mo][:, :],
                                         w1es[:, mo * 128:(mo + 1) * 128],
                                         rhs, start=True, stop=True)
                    for mo in range(2):
                        if j in ENC_L1_DVE:
                            nc.vector._custom_dve(LRELU_OP, out=h1[:, mo, :],
                                                  in0=p1[mo][:, :], s0=0.01)
                        else:
                            nc.scalar.activation(h1[:, mo, :], p1[mo][:, :],
                                                 AF.Prelu, alpha=0.01)
                    h2 = sp.tile([128, 2, 512], BF16, tag="h2")
                    for mo in range(2):
                        p2 = pp.tile([128, 512], F32, tag=f"pB{mo}", bufs=1)
                        for ki in range(2):
                            nc.tensor.matmul(p2[:, :],
                                             w2_sb[:, ki * 2 + mo, :],
                                             h1[:, ki, :],
                                             start=(ki == 0), stop=(ki == 1))
                        if zb and j in ENC_L2_DVE:
                            nc.vector._custom_dve(LRELU_OP, out=h2[:, mo, :],
                                                  in0=p2[:, :], s0=0.01)
                        elif zb:
                            nc.scalar.activation(h2[:, mo, :], p2[:, :],
                                                 AF.Prelu, alpha=0.01)
                        else:
                            nc.scalar.activation(h2[:, mo, :], p2[:, :],
                                                 AF.Prelu,
                                                 bias=b2c_sb[:, mo:mo + 1],
                                                 alpha=0.01)
                    for ki in range(2):
                        nc.tensor.matmul(p3[jj * 64:jj * 64 + 64, :],
                                         w3_sb[:, ki, :], h2[:, ki, :],
                                         start=(ki == 0), stop=(ki == 1))
                yt = sp.tile([128, 512], BF16, tag="yt")
                nc.vector.tensor_copy(yt[:], p3[:, :])
                nc.gpsimd.dma_start(out=yencP_o[jb, :, :], in_=yt[:])
                return yt

            yt0 = encoder_jb(0)
            nc.vector.tensor_copy(y0S, yt0[0:LDIM, :])
            nc.sync.dma_start_transpose(yT, y0S[:])
            # yT[:, c, 0:32] = y00, yT[:, c, 32:64] = y01

            # ------------- rotation (2 groups, interleaved) -------------
            def rot_group(cs):
                angs, rads, ees, sss, ccs = {}, {}, {}, {}, {}
                outs = {}
                for c in cs:
                    om = mT[:, c, 32:64].unsqueeze(1) \
                        .broadcast_to([128, T, 32])
                    mu = mT[:, c, 0:32].unsqueeze(1) \
                        .broadcast_to([128, T, 32])
                    ang = rp.tile([128, T, 32], BF16, tag="ang", bufs=2)
                    nc.vector.tensor_mul(ang, tvk_sb[:], om)
                    rad = rp.tile([128, T, 32], BF16, tag="rad", bufs=2)
                    nc.vector.tensor_mul(rad, tvk_sb[:], mu)
                    angs[c], rads[c] = ang, rad
                for c in cs:       # batched: one Exp table load
                    ee = rp.tile([128, T, 32], BF16, tag="ee", bufs=2)
                    nc.scalar.activation(ee, rads[c][:], AF.Exp)
                    ees[c] = ee
                for c in cs:       # batched: one Sin table load
                    ss = rp.tile([128, T, 32], BF16, tag="ss", bufs=2)
                    nc.scalar.activation(ss, angs[c][:], AF.Sin)
                    cc = rp.tile([128, T, 32], BF16, tag="cc", bufs=2)
                    nc.scalar.activation(cc, angs[c][:], AF.Sin,
                                         bias=hpib[:, :], scale=-1.0)
                    sss[c], ccs[c] = ss, cc
                for c in cs:
                    y00 = yT[:, c, 0:32].unsqueeze(1) \
                        .broadcast_to([128, T, 32])
                    y01 = yT[:, c, 32:64].unsqueeze(1) \
                        .broadcast_to([128, T, 32])
                    ec = rp.tile([128, T, 32], BF16, tag="ec", bufs=2)
                    nc.gpsimd.tensor_mul(ec, ees[c][:], ccs[c][:])
                    es = rp.tile([128, T, 32], BF16, tag="es", bufs=2)
                    nc.vector.tensor_mul(es, ees[c][:], sss[c][:])
                    t2 = rp.tile([128, T, 32], BF16, tag="t2", bufs=1)
                    nc.vector.tensor_mul(t2, es[:], y01)
                    t3 = rp.tile([128, T, 32], BF16, tag="t3", bufs=1)
                    nc.vector.tensor_mul(t3, es[:], y00)
                    t1 = rp.tile([128, T, 32], BF16, tag="t1", bufs=1)
                    nc.vector.tensor_mul(t1, ec[:], y00)
                    o0 = rp.tile([128, T, 32], BF16, tag="o0", bufs=2)
                    nc.vector.tensor_sub(o0, t1[:], t2[:])
                    t4 = rp.tile([128, T, 32], BF16, tag="t4", bufs=1)
                    nc.vector.tensor_mul(t4, ec[:], y01)
                    o1 = rp.tile([128, T, 32], BF16, tag="o1", bufs=2)
                    nc.vector.tensor_add(o1, t3[:], t4[:])
                    nc.gpsimd.dma_start(out=yl0_o[c, :, :, :], in_=o0[:])
                    nc.gpsimd.dma_start(out=yl1_o[c, :, :, :], in_=o1[:])
                    outs[c] = (o0, o1)
                return outs

            def xpred(c, o0, o1):
                xin0 = rp.tile([128, 16, 128], BF16, tag="xin0", bufs=2)
                nc.sync.dma_start_transpose(
                    xin0, o0[:].rearrange("p t k -> p (t k)"))
                xin1 = rp.tile([128, 16, 128], BF16, tag="xin1", bufs=2)
                nc.sync.dma_start_transpose(
                    xin1, o1[:].rearrange("p t k -> p (t k)"))
                for h in range(2):
                    pxp = pp.tile([128, 512], F32, tag="pC", bufs=2)
                    for qq in range(2):
                        q = h * 2 + qq
                        sl = slice(64 * qq, 64 * qq + 64)
                        nc.tensor.matmul(pxp[sl, :], cwe_sb[:],
                                         xin0[:, 4 * q:4 * q + 4, :],
                                         start=True, stop=False)
                        nc.tensor.matmul(pxp[sl, :], cwo_sb[:],
                                         xin1[:, 4 * q:4 * q + 4, :],
                                         start=False, stop=True)
                    xpt = sp.tile([128, 512], BF16, tag="xpt")
                    nc.vector.tensor_copy(xpt[:], pxp[:, :])
                    nc.sync.dma_start(out=xp_o[c, h, :, :], in_=xpt[:])

            for jb in range(1, 14):
                encoder_jb(jb)
            og0 = rot_group([0, 1])
            for jb in range(14, 16):
                encoder_jb(jb)
            xpred(0, *og0[0])
            for jb in range(16, 18):
                encoder_jb(jb)
            xpred(1, *og0[1])
            for jb in range(18, 22):
                encoder_jb(jb)
            og1 = rot_group([2, 3])
            for jb in range(22, 24):
                encoder_jb(jb)
            xpred(2, *og1[2])
            for jb in range(24, 26):
                encoder_jb(jb)
            xpred(3, *og1[3])
            for jb in range(26, 32):
                encoder_jb(jb)
            rp.release()
    nc.finalize()
    return nc


def _host_prep(inputs):
    f32, bf = np.float32, ml_dtypes.bfloat16
    xs = np.asarray(inputs["xs"], f32)
    perm = np.concatenate([np.arange(0, LDIM, 2), np.arange(1, LDIM, 2)])

    w1g = np.asarray(inputs["enc_W1"], f32) * np.asarray(inputs["enc_bn_gamma"], f32)
    w1e = np.ascontiguousarray(w1g.T)                       # [16, 256] f32
    b1h = (np.asarray(inputs["enc_b1"], f32)
           + np.asarray(inputs["enc_W1"], f32) @ np.asarray(inputs["enc_bn_beta"], f32))
    w2 = np.asarray(inputs["enc_W2"], f32)
    w2sb = np.empty((128, 4, 128), f32)
    for ki in range(2):
        for mo in range(2):
            w2sb[:, ki * 2 + mo, :] = w2[mo * 128:(mo + 1) * 128,
                                         ki * 128:(ki + 1) * 128].T
    b2 = np.asarray(inputs["enc_b2"], f32)
    b2c = np.ascontiguousarray(b2.reshape(2, 128).T)        # [128, 2]
    w3p = (np.asarray(inputs["enc_W3"], f32)
           * np.asarray(inputs["enc_scale"], f32)[:, None])[perm]   # [64, 256]
    w3sb = np.empty((128, 2, LDIM), f32)
    for ki in range(2):
        w3sb[:, ki, :] = w3p[:, ki * 128:(ki + 1) * 128].T

    w1a = (np.asarray(inputs["aux_W1"], f32)
           * np.asarray(inputs["aux_bn_gamma"], f32)[:, None, :])
    w1asb = np.ascontiguousarray(w1a.reshape(NAUX * AH, DIM).T)     # [16, 4096]
    b1a = (np.asarray(inputs["aux_b1"], f32)
           + np.einsum("kji,ki->kj", np.asarray(inputs["aux_W1"], f32),
                       np.asarray(inputs["aux_bn_beta"], f32)))
    b2a = np.asarray(inputs["aux_b2"], f32)
    w2asb = np.ascontiguousarray(
        np.asarray(inputs["aux_W2"], f32).transpose(2, 0, 1))       # [128, 32, 128]
    b2ac = np.ascontiguousarray(b2a.T)                              # [128, 32]
    w3adt = (np.asarray(inputs["aux_W3"], f32)
             * np.asarray(inputs["aux_scale"], f32)[:, :, None] * DT)
    w3asb = np.zeros((AH, NAUX, LDIM), f32)
    for k in range(NAUX):
        w3asb[:, k, k] = w3adt[k, 0, :]          # mu -> row k
        w3asb[:, k, 32 + k] = w3adt[k, 1, :]     # om -> row 32+k
    cw = np.asarray(inputs["Cw"], f32)                      # [16, 64]
    cwE, cwO = cw[:, 0::2], cw[:, 1::2]                     # [16, 32]
    cwe = np.zeros((128, 64), f32)
    cwo = np.zeros((128, 64), f32)
    for tsub in range(4):
        cwe[tsub * 32:(tsub + 1) * 32, tsub * 16:(tsub + 1) * 16] = cwE.T
        cwo[tsub * 32:(tsub + 1) * 32, tsub * 16:(tsub + 1) * 16] = cwO.T
    tvk = np.broadcast_to(np.arange(T, dtype=f32)[None, :, None],
                          (128, T, 32))

    shared = dict(
        w1e=w1e, b1r=np.ascontiguousarray(b1h.reshape(1, H)),
        w2=w2sb.astype(bf), b2c=b2c, w3=w3sb.astype(bf),
        w1a=w1asb, b1ar=b1a.reshape(1, NAUX * AH).astype(bf),
        w2a=w2asb.astype(bf), b2ac=b2ac, w3a=w3asb.astype(bf),
        cwe=cwe.astype(bf), cwo=cwo.astype(bf),
        tvk=np.ascontiguousarray(tvk).astype(bf),
        id16=np.eye(DIM, dtype=f32),
        onesr=np.ones((1, BC), f32).astype(bf))
    zb = bool(np.all(b2 == 0.0) and np.all(b2a == 0.0))
    in_maps = []
    for c in range(NCORES):
        xc = xs[c * BC:(c + 1) * BC]                        # [512, 64, 16]
        xsT = np.empty((D1, T, BC), f32)
        xsT[0:DIM] = xc.transpose(2, 1, 0)
        xsT[DIM] = 1.0
        xsN = np.concatenate(
            [xc.reshape(COLS, DIM), np.ones((COLS, 1), f32),
             np.zeros((COLS, 1), f32)], axis=1)             # [32768, 18]
        # partition-major packing: row (g*128 + p) -> xsNg[p, g, :]
        xsNg = np.ascontiguousarray(
            xsN.reshape(256, 128, 18).transpose(1, 0, 2))
        x0r = xsN.reshape(BC, T, 18)[:, 0, :]               # [512, 18]
        xsN0 = np.ascontiguousarray(x0r.reshape(4, 128, 18).transpose(1, 0, 2))
        m = dict(shared)
        m["xsT"] = xsT.astype(bf)
        m["xsNg"] = xsNg.astype(bf)
        m["xsN0"] = xsN0.astype(bf)
        in_maps.append(m)
    return in_maps, zb


def _assemble(inputs, results):
    f32 = np.float32
    xs = np.asarray(inputs["xs"], f32)
    perm = np.concatenate([np.arange(0, LDIM, 2), np.arange(1, LDIM, 2)])
    y = np.empty((B, T, DIM + LDIM), f32)
    y_pred = np.empty((B, T, DIM + LDIM), f32)
    y[:, :, :DIM] = xs
    for c in range(NCORES):
        r = results[c]
        sl = slice(c * BC, (c + 1) * BC)
        ye = np.asarray(r["yencP"], f32).reshape(32, 2, LDIM, BC)
        yenc_dev = ye.transpose(3, 0, 1, 2).reshape(BC, T, LDIM)
        y[sl, :, 16 + perm] = yenc_dev
        yl0 = np.asarray(r["yl0"], f32).reshape(BC, T, 32)
        yl1 = np.asarray(r["yl1"], f32).reshape(BC, T, 32)
        ylf = np.empty((BC, T, LDIM), f32)
        ylf[:, :, 0::2] = yl0
        ylf[:, :, 1::2] = yl1
        y_pred[sl, :, DIM:] = ylf
        # xp [4, 2, 128, 512] -> [cc, h, (qq, tsub, i), (jt, bsub)]
        xpr = np.asarray(r["xp"], f32).reshape(4, 2, 2, 4, 16, 4, 128)
        xpr = xpr.transpose(0, 6, 1, 2, 5, 3, 4).reshape(BC, T, DIM)
        y_pred[sl, :, :DIM] = xpr
    y_pred[:, 0, :DIM] = xs[:, 0, :]
    return y, y_pred


_NC_CACHE = {}


def kernel(**inputs):
    in_maps, zb = _host_prep(inputs)
    key = ("nc", zb)
    if key not in _NC_CACHE:
        _NC_CACHE[key] = build(zb=zb)
    nc = _NC_CACHE[key]
    res = bass_utils.run_bass_kernel_spmd(nc, in_maps,
                                          core_ids=list(range(NCORES)))
    return _assemble(inputs, res.results)


# revision 53
# speedup vs baseline: 1.0026x; 1.0026x over previous
"""Trainium2 Bass kernel for nn_DENIS_JBF (Koopman Jordan-block forecast).

v3 strategy (pure data parallel, 8 cores, Bc = 512 per core), all-bf16:
  - BN stats: host-packed partition-major Gram input (one DMA), bf16 PE
    Gram matmuls, AllGather (cheaper than AllReduce) + 3-step local
    reduce; BN folded into L1 weights on device.
  - One-pass leaky-relu evictions: ACT Prelu(alpha) from PSUM with L2
    bias fused (or skipped entirely when the biases are all zero, per
    input_specs); a tunable subset of tiles goes to DVE (copy+STT) to
    balance engines.
  - All-bf16: PE 1 col/cycle, DVE 2x_1p fast mode, half-size outputs.
  - Rotation in closed form, [128b, t, k] tiles with in-op broadcasts;
    Sin/Exp batched to avoid ACT table thrash; ec/es on GPSIMD (Pool).
  - yenc dims permuted (even/odd split) via host-permuted enc_W3 so
    mu/om and y0 pairs are contiguous after one blocked DMA-transpose.
  - x_pred: o0/o1 blocked-DMA-transposed, block-diagonal Cw matmuls.
  - DMA queues by role: SP = loads (deep xst prefetch) + stats + xp;
    GPSIMD/SWDGE = bulk stores (yencP, yl); ACT = dma transposes.
"""

import os
import sys

import numpy as np
import ml_dtypes

for _p in ("/opt/trn_rl_repo", "/root/.axon_site/_ro/trn_rl_repo"):
    if os.path.isdir(_p) and _p not in sys.path:
        sys.path.insert(0, _p)

import concourse.bass as bass
from concourse import bacc
import concourse.mybir as mybir
import concourse.tile as tile
from concourse import bass_utils
import concourse.dve_ops as dve_ops
from concourse.dve_ops import DveOp
from concourse.dve_spec import Spec, Src0, C0, maxx, lower
from concourse.dve_uop import DveOpSpec


def _register_lrelu():
    """Single-pass leaky-relu as a custom DVE op: max(Src0*C0, Src0)."""
    if "LRELU_ANT" in dve_ops._SUB_OPCODE_FOR_NAME:
        return next(op for op in dve_ops.OPS if op.name == "LRELU_ANT")
    spec = Spec(
        body=maxx(Src0 * C0, Src0),
        reference=lambda in0, in1, s0, s1, imm2: np.maximum(
            in0.astype(np.float32) * s0, in0.astype(np.float32)),
    )
    row = dve_ops._CUSTOM_DVE_ROW_BASE + len(dve_ops.OPS)
    assert row < 0x20
    dve_ops._SUB_OPCODE_FOR_NAME["LRELU_ANT"] = row
    shas = {}
    for ver in ("v3", "v4"):
        uops = lower(spec, ver=ver)
        shas[ver] = DveOpSpec(name="LRELU_ANT", opcode=row, uops=uops,
                              rd1_en=False).sha(ver)
    op = DveOp("LRELU_ANT", spec, subdim=False, uops_sha=shas)
    dve_ops.OPS.append(op)
    dve_ops.CUSTOM_DVE_SPECS["LRELU_ANT"] = spec
    return op


LRELU_OP = _register_lrelu()

F32 = mybir.dt.float32
BF16 = mybir.dt.bfloat16
AF = mybir.ActivationFunctionType
OP = mybir.AluOpType
AX = mybir.AxisListType

NCORES = 8
B, T, DIM, LDIM, NAUX = 4096, 64, 16, 64, 32
H, AH = 256, 128
DT = 0.01
EPS = 1e-5
BC = B // NCORES            # 512
COLS = BC * T               # 32768, col = t*BC + b
D1 = DIM + 1                # 17: +ones row (L1 bias in matmul)
PI = float(np.pi)
PF = 8                      # xst prefetch depth

# --- engine-assignment tunables -----------------------------------------
ENC_L1_DVE = frozenset(j for j in range(64) if j % 2 == 1)
ENC_L2_DVE = frozenset(j for j in range(64) if j % 4 == 1)
AUX_L1_DVE = frozenset(range(16))


def build(zb=True):
    """zb: biases (enc_b2 / aux_b2) are all-zero -> single-op L2 evictions."""
    nc = bacc.Bacc(None)

    # ---------------- DRAM I/O ----------------
    xsT_d = nc.dram_tensor("xsT", [D1, T, BC], BF16, kind="ExternalInput")
    xsNg_d = nc.dram_tensor("xsNg", [128, 256, 18], BF16, kind="ExternalInput")
    xsN0_d = nc.dram_tensor("xsN0", [128, 4, 18], BF16, kind="ExternalInput")
    w1e_d = nc.dram_tensor("w1e", [DIM, H], F32, kind="ExternalInput")
    b1r_d = nc.dram_tensor("b1r", [1, H], F32, kind="ExternalInput")
    w2_d = nc.dram_tensor("w2", [128, 4, 128], BF16, kind="ExternalInput")
    b2c_d = nc.dram_tensor("b2c", [128, 2], F32, kind="ExternalInput")
    w3_d = nc.dram_tensor("w3", [128, 2, LDIM], BF16, kind="ExternalInput")
    w1a_d = nc.dram_tensor("w1a", [DIM, NAUX * AH], F32, kind="ExternalInput")
    b1ar_d = nc.dram_tensor("b1ar", [1, NAUX * AH], BF16, kind="ExternalInput")
    w2a_d = nc.dram_tensor("w2a", [AH, NAUX, AH], BF16, kind="ExternalInput")
    b2ac_d = nc.dram_tensor("b2ac", [128, NAUX], F32, kind="ExternalInput")
    w3a_d = nc.dram_tensor("w3a", [AH, NAUX, LDIM], BF16, kind="ExternalInput")
    cwe_d = nc.dram_tensor("cwe", [128, 64], BF16, kind="ExternalInput")
    cwo_d = nc.dram_tensor("cwo", [128, 64], BF16, kind="ExternalInput")
    tvk_d = nc.dram_tensor("tvk", [128, T, 32], BF16, kind="ExternalInput")
    id16_d = nc.dram_tensor("id16", [DIM, DIM], F32, kind="ExternalInput")
    onesr_d = nc.dram_tensor("onesr", [1, BC], BF16, kind="ExternalInput")

    yencP_o = nc.dram_tensor("yencP", [32, 128, BC], BF16, kind="ExternalOutput")
    yl0_o = nc.dram_tensor("yl0", [4, 128, T, 32], BF16, kind="ExternalOutput")
    yl1_o = nc.dram_tensor("yl1", [4, 128, T, 32], BF16, kind="ExternalOutput")
    xp_o = nc.dram_tensor("xp", [4, 2, 128, 512], BF16, kind="ExternalOutput")

    stat_in = nc.dram_tensor("stat_in", [2, 18, 18], F32)
    stat_out = nc.dram_tensor("stat_out", [NCORES, 2, 18, 18], F32,
                              addr_space="Shared")

    with tile.TileContext(nc) as tc:
        with tc.tile_pool(name="consts", bufs=1) as cp, \
             tc.tile_pool(name="psum", bufs=1, space="PSUM") as pp, \
             tc.tile_pool(name="stream", bufs=2) as sp, \
             tc.tile_pool(name="smalls", bufs=1) as smp:
            bp = tc.alloc_tile_pool(name="boot", bufs=1)

            # ------------- weights / consts (SP loads, overlap stats) ----
            def ld(shape, dt, dram, name):
                t = cp.tile(shape, dt, tag=name)
                nc.sync.dma_start(out=t, in_=dram)
                return t

            # stats inputs first: the Gram + AllGather is the serial prologue
            xg = bp.tile([128, 256, 18], BF16, tag="xg", bufs=1)
            nc.sync.dma_start(out=xg, in_=xsNg_d[:, :, :])
            x0g = bp.tile([128, 4, 18], BF16, tag="x0g", bufs=1)
            nc.sync.dma_start(out=x0g, in_=xsN0_d[:, :, :])
            w1e_sb = ld([DIM, H], F32, w1e_d[:, :], "w1e")
            b1r_sb = ld([1, H], F32, b1r_d[:, :], "b1r")
            w2_sb = ld([128, 4, 128], BF16, w2_d[:, :, :], "w2")
            b2c_sb = ld([128, 2], F32, b2c_d[:, :], "b2c")
            w3_sb = ld([128, 2, LDIM], BF16, w3_d[:, :, :], "w3")
            w2a_sb = ld([AH, NAUX, AH], BF16, w2a_d[:, :, :], "w2a")
            b2ac_sb = ld([128, NAUX], F32, b2ac_d[:, :], "b2ac")
            w3a_sb = ld([AH, NAUX, LDIM], BF16, w3a_d[:, :, :], "w3a")
            cwe_sb = ld([128, 64], BF16, cwe_d[:, :], "cwe")
            cwo_sb = ld([128, 64], BF16, cwo_d[:, :], "cwo")
            tvk_sb = ld([128, T, 32], BF16, tvk_d[:, :, :], "tvk")
            id16_sb = ld([DIM, DIM], F32, id16_d[:, :], "id16")
            w1a_sb = bp.tile([DIM, NAUX * AH], F32, tag="w1atmp", bufs=1)
            nc.sync.dma_start(out=w1a_sb, in_=w1a_d[:, :])
            x0T = bp.tile([DIM, BC], BF16, tag="x0T", bufs=1)
            nc.sync.dma_start(out=x0T, in_=xsT_d[0:DIM, 0, :])

            hpib = cp.tile([128, 1], F32)
            nc.vector.memset(hpib, PI / 2.0)
            epsb = cp.tile([128, 1], F32)
            nc.vector.memset(epsb, EPS)

            # xst stream with deep prefetch (pure loads on SP queue)
            def load_xst(jb):
                xst = sp.tile([D1, 2, 512], BF16, tag="xst", bufs=PF)
                nc.sync.dma_start(
                    out=xst, in_=xsT_d[:, jb * 2:(jb + 1) * 2, :])
                return xst

            xst_tiles = {}
            for jb in range(min(PF, 32)):
                xst_tiles[jb] = load_xst(jb)

            # ------------- phase A: Gram stats + AllGather -------------
            pg0 = pp.tile([18, 18], F32, tag="pC", bufs=2)
            for g in range(4):
                nc.tensor.matmul(pg0[:, :], x0g[:, g, :], x0g[:, g, :],
                                 start=(g == 0), stop=(g == 3))
            pg = pp.tile([18, 18], F32, tag="pA0", bufs=2)
            for g in range(256):
                nc.tensor.matmul(pg[:, :], xg[:, g, :], xg[:, g, :],
                                 start=(g == 0), stop=(g == 255))
            gA = smp.tile([18, 18], F32)
            nc.vector.tensor_copy(gA, pg[:, :])
            gB = smp.tile([18, 18], F32)
            nc.vector.tensor_copy(gB, pg0[:, :])
            nc.gpsimd.dma_start(out=stat_in[0, :, :], in_=gA[:])
            nc.gpsimd.dma_start(out=stat_in[1, :, :], in_=gB[:])
            nc.gpsimd.collective_compute(
                "AllGather", OP.bypass, replica_groups=[list(range(NCORES))],
                ins=[stat_in[:, :, :]], outs=[stat_out[:, :, :, :]])
            g8 = smp.tile([18, NCORES, 2, 18], F32)
            nc.gpsimd.dma_start(out=g8,
                                in_=stat_out[:, :, :, :].transpose([2, 0, 1, 3]))
            r4 = smp.tile([18, 4, 2, 18], F32)
            nc.vector.tensor_add(r4, g8[:, 0:4, :, :], g8[:, 4:8, :, :])
            r2 = smp.tile([18, 2, 2, 18], F32)
            nc.vector.tensor_add(r2, r4[:, 0:2, :, :], r4[:, 2:4, :, :])
            stats = smp.tile([18, 2, 18], F32)
            nc.vector.tensor_add(stats, r2[:, 0, :, :], r2[:, 1, :, :])

            # ------------- phase F: fold BN into L1 weights -------------
            def fold(set_idx, n):
                g16 = stats[0:DIM, set_idx, 0:DIM]
                scol = stats[0:DIM, set_idx, DIM:DIM + 1]
                m = smp.tile([DIM, 1], F32, tag=f"m{set_idx}")
                nc.vector.tensor_scalar(m, scol, 1.0 / n, None, OP.mult)
                gi = smp.tile([DIM, DIM], F32, tag=f"gi{set_idx}")
                nc.vector.tensor_mul(gi, g16, id16_sb[:])
                qd = smp.tile([DIM, 1], F32, tag=f"qd{set_idx}")
                nc.vector.reduce_sum(qd, gi, axis=AX.X)
                m2 = smp.tile([DIM, 1], F32, tag=f"m2{set_idx}")
                nc.vector.tensor_mul(m2, m, m)
                v = smp.tile([DIM, 1], F32, tag=f"v{set_idx}")
                nc.vector.scalar_tensor_tensor(v, qd, 1.0 / n, m2,
                                               OP.mult, OP.subtract)
                # rs = (v+eps)^-1/2 via Ln+Exp (one shared act table set)
                lnv = smp.tile([DIM, 1], F32, tag=f"ln{set_idx}")
                nc.scalar.activation(lnv, v, AF.Ln, bias=epsb[0:DIM, :])
                rs = smp.tile([DIM, 1], F32, tag=f"rs{set_idx}")
                nc.scalar.activation(rs, lnv, AF.Exp, scale=-0.5)
                return m, rs

            m_all, rs_all = fold(0, float(B * T))
            m_0, rs_0 = fold(1, float(B))

            w1es = cp.tile([D1, H], BF16)
            nc.vector.tensor_scalar(w1es[0:DIM, :], w1e_sb[:], rs_all[:],
                                    None, OP.mult)
            mneg = smp.tile([DIM, 1], BF16)
            nc.vector.tensor_scalar(mneg, m_all[:], -1.0, None, OP.mult)
            pbc = pp.tile([1, H], F32, tag="pC", bufs=2)
            nc.tensor.matmul(pbc[:, :], mneg[:], w1es[0:DIM, :],
                             start=True, stop=True)
            badd = smp.tile([1, H], BF16)
            nc.vector.tensor_add(badd, pbc[:, :], b1r_sb[:])
            nc.sync.dma_start(out=w1es[DIM:D1, :], in_=badd[:])

            w1as = cp.tile([D1, NAUX * AH], BF16)
            nc.vector.tensor_scalar(w1as[0:DIM, 0:2048], w1a_sb[:, 0:2048],
                                    rs_0[:], None, OP.mult)
            nc.scalar.activation(w1as[0:DIM, 2048:4096],
                                 w1a_sb[:, 2048:4096], AF.Copy,
                                 scale=rs_0[:])
            nc.sync.dma_start(out=w1as[DIM:D1, :], in_=b1ar_d[:, :])

            m0n = smp.tile([DIM, 1], F32)
            nc.vector.tensor_scalar(m0n, m_0[:], -1.0, None, OP.mult)
            x0c = cp.tile([D1, BC], BF16)
            nc.vector.tensor_scalar(x0c[0:DIM, :], x0T[:], m0n[:],
                                    None, OP.add)
            nc.sync.dma_start(out=x0c[DIM:D1, :], in_=onesr_d[:, :])
            bp.release()
            rp = tc.alloc_tile_pool(name="rot", bufs=1)

            # ------------- phase X: aux nets -> pmw [64, 512] -------------
            pmw = pp.tile([LDIM, BC], F32, tag="pC", bufs=2)
            for kp in range(NAUX // 2):
                h1a = sp.tile([128, 2, BC], BF16, tag="h1a", bufs=2)
                pa1 = [pp.tile([128, BC], F32, tag=f"pA{s}", bufs=2,
                               name=f"pa1_{s}") for s in range(2)]
                for s in range(2):
                    k = kp * 2 + s
                    nc.tensor.matmul(pa1[s][:, :],
                                     w1as[:, k * AH:(k + 1) * AH],
                                     x0c[:], start=True, stop=True)
                for s in range(2):
                    if kp in AUX_L1_DVE:
                        nc.vector._custom_dve(LRELU_OP, out=h1a[:, s, :],
                                              in0=pa1[s][:, :], s0=0.01)
                    else:
                        nc.scalar.activation(h1a[:, s, :], pa1[s][:, :],
                                             AF.Prelu, alpha=0.01)
                h2a = sp.tile([128, 2, BC], BF16, tag="h2a", bufs=2)
                for s in range(2):
                    k = kp * 2 + s
                    pa2 = pp.tile([128, BC], F32, tag=f"pB{s}", bufs=1)
                    nc.tensor.matmul(pa2[:, :], w2a_sb[:, k, :],
                                     h1a[:, s, :], start=True, stop=True)
                    if zb:
                        nc.scalar.activation(h2a[:, s, :], pa2[:, :],
                                             AF.Prelu, alpha=0.01)
                    else:
                        nc.scalar.activation(h2a[:, s, :], pa2[:, :],
                                             AF.Prelu,
                                             bias=b2ac_sb[:, k:k + 1],
                                             alpha=0.01)
                    nc.tensor.matmul(pmw[:, :], w3a_sb[:, k, :],
                                     h2a[:, s, :],
                                     start=(k == 0), stop=(k == NAUX - 1))

            # muom -> SBUF bf16, blocked transpose to b-major
            muomS = cp.tile([LDIM, BC], BF16)
            nc.vector.tensor_copy(muomS, pmw[:, :])
            mT = cp.tile([128, 4, LDIM], BF16)
            nc.sync.dma_start_transpose(mT, muomS[:])
            # mT[:, c, 0:32] = mu, mT[:, c, 32:64] = om

            # ------------- phase E: encoder -------------
            y0S = cp.tile([LDIM, BC], BF16)
            yT = cp.tile([128, 4, LDIM], BF16)

            def encoder_jb(jb):
                if jb + PF < 32:
                    xst_tiles[jb + PF] = load_xst(jb + PF)
                xst = xst_tiles.pop(jb)
                p3 = pp.tile([128, 512], F32, tag="pC", bufs=2)
                for jj in range(2):
                    j = jb * 2 + jj
                    rhs = xst[:, jj, :]
                    h1 = sp.tile([128, 2, 512], BF16, tag="h1")
                    p1 = [pp.tile([128, 512], F32, tag=f"pA{mo}", bufs=2,
                                  name=f"p1_{mo}") for mo in range(2)]
                    for mo in range(2):
                        nc.tensor.matmul(p1[mo][:, :],
                                         w1es[:, mo * 128:(mo + 1) * 128],
                                         rhs, start=True, stop=True)
                    for mo in range(2):
                        if j in ENC_L1_DVE:
                            nc.vector._custom_dve(LRELU_OP, out=h1[:, mo, :],
                                                  in0=p1[mo][:, :], s0=0.01)
                        else:
                            nc.scalar.activation(h1[:, mo, :], p1[mo][:, :],
                                                 AF.Prelu, alpha=0.01)
                    h2 = sp.tile([128, 2, 512], BF16, tag="h2")
                    for mo in range(2):
                        p2 = pp.tile([128, 512], F32, tag=f"pB{mo}", bufs=1)
                        for ki in range(2):
                            nc.tensor.matmul(p2[:, :],
                                             w2_sb[:, ki * 2 + mo, :],
                                             h1[:, ki, :],
                                             start=(ki == 0), stop=(ki == 1))
                        if zb and j in ENC_L2_DVE:
                            nc.vector._custom_dve(LRELU_OP, out=h2[:, mo, :],
                                                  in0=p2[:, :], s0=0.01)
                        elif zb:
                            nc.scalar.activation(h2[:, mo, :], p2[:, :],
                                                 AF.Prelu, alpha=0.01)
                        else:
                            nc.scalar.activation(h2[:, mo, :], p2[:, :],
                                                 AF.Prelu,
                                                 bias=b2c_sb[:, mo:mo + 1],
                                                 alpha=0.01)
                        nc.tensor.matmul(p3[jj * 64:jj * 64 + 64, :],
                                         w3_sb[:, mo, :], h2[:, mo, :],
                                         start=(mo == 0), stop=(mo == 1))
                yt = sp.tile([128, 512], BF16, tag="yt")
                nc.vector.tensor_copy(yt[:], p3[:, :])
                nc.gpsimd.dma_start(out=yencP_o[jb, :, :], in_=yt[:])
                return yt

            yt0 = encoder_jb(0)
            nc.vector.tensor_copy(y0S, yt0[0:LDIM, :])
            nc.sync.dma_start_transpose(yT, y0S[:])
            # yT[:, c, 0:32] = y00, yT[:, c, 32:64] = y01

            # ------------- rotation (2 groups, interleaved) -------------
            def rot_group(cs):
                angs, rads, ees, sss, ccs = {}, {}, {}, {}, {}
                outs = {}
                for c in cs:
                    om = mT[:, c, 32:64].unsqueeze(1) \
                        .broadcast_to([128, T, 32])
                    mu = mT[:, c, 0:32].unsqueeze(1) \
                        .broadcast_to([128, T, 32])
                    ang = rp.tile([128, T, 32], BF16, tag="ang", bufs=2)
                    nc.vector.tensor_mul(ang, tvk_sb[:], om)
                    rad = rp.tile([128, T, 32], BF16, tag="rad", bufs=2)
                    nc.vector.tensor_mul(rad, tvk_sb[:], mu)
                    angs[c], rads[c] = ang, rad
                for c in cs:       # batched: one Exp table load
                    ee = rp.tile([128, T, 32], BF16, tag="ee", bufs=2)
                    nc.scalar.activation(ee, rads[c][:], AF.Exp)
                    ees[c] = ee
                for c in cs:       # batched: one Sin table load
                    ss = rp.tile([128, T, 32], BF16, tag="ss", bufs=2)
                    nc.scalar.activation(ss, angs[c][:], AF.Sin)
                    cc = rp.tile([128, T, 32], BF16, tag="cc", bufs=2)
                    nc.scalar.activation(cc, angs[c][:], AF.Sin,
                                         bias=hpib[:, :], scale=-1.0)
                    sss[c], ccs[c] = ss, cc
                for c in cs:
                    y00 = yT[:, c, 0:32].unsqueeze(1) \
                        .broadcast_to([128, T, 32])
                    y01 = yT[:, c, 32:64].unsqueeze(1) \
                        .broadcast_to([128, T, 32])
                    ec = rp.tile([128, T, 32], BF16, tag="ec", bufs=2)
                    nc.gpsimd.tensor_mul(ec, ees[c][:], ccs[c][:])
                    es = rp.tile([128, T, 32], BF16, tag="es", bufs=2)
                    nc.vector.tensor_mul(es, ees[c][:], sss[c][:])
                    t2 = rp.tile([128, T, 32], BF16, tag="t2", bufs=1)
                    nc.vector.tensor_mul(t2, es[:], y01)
                    t3 = rp.tile([128, T, 32], BF16, tag="t3", bufs=1)
                    nc.vector.tensor_mul(t3, es[:], y00)
                    t1 = rp.tile([128, T, 32], BF16, tag="t1", bufs=1)
                    nc.vector.tensor_mul(t1, ec[:], y00)
                    o0 = rp.tile([128, T, 32], BF16, tag="o0", bufs=2)
                    nc.vector.tensor_sub(o0, t1[:], t2[:])
                    t4 = rp.tile([128, T, 32], BF16, tag="t4", bufs=1)
                    nc.vector.tensor_mul(t4, ec[:], y01)
                    o1 = rp.tile([128, T, 32], BF16, tag="o1", bufs=2)
                    nc.vector.tensor_add(o1, t3[:], t4[:])
                    nc.gpsimd.dma_start(out=yl0_o[c, :, :, :], in_=o0[:])
                    nc.gpsimd.dma_start(out=yl1_o[c, :, :, :], in_=o1[:])
                    outs[c] = (o0, o1)
                return outs

            def xpred(c, o0, o1):
                xin0 = rp.tile([128, 16, 128], BF16, tag="xin0", bufs=2)
                nc.sync.dma_start_transpose(
                    xin0, o0[:].rearrange("p t k -> p (t k)"))
                xin1 = rp.tile([128, 16, 128], BF16, tag="xin1", bufs=2)
                nc.sync.dma_start_transpose(
                    xin1, o1[:].rearrange("p t k -> p (t k)"))
                for h in range(2):
                    pxp = pp.tile([128, 512], F32, tag="pC", bufs=2)
                    for qq in range(2):
                        q = h * 2 + qq
                        sl = slice(64 * qq, 64 * qq + 64)
                        nc.tensor.matmul(pxp[sl, :], cwe_sb[:],
                                         xin0[:, 4 * q:4 * q + 4, :],
                                         start=True, stop=False)
                        nc.tensor.matmul(pxp[sl, :], cwo_sb[:],
                                         xin1[:, 4 * q:4 * q + 4, :],
                                         start=False, stop=True)
                    xpt = sp.tile([128, 512], BF16, tag="xpt")
                    nc.vector.tensor_copy(xpt[:], pxp[:, :])
                    nc.sync.dma_start(out=xp_o[c, h, :, :], in_=xpt[:])

            for jb in range(1, 14):
                encoder_jb(jb)
            og0 = rot_group([0, 1])
            for jb in range(14, 16):
                encoder_jb(jb)
            xpred(0, *og0[0])
            for jb in range(16, 18):
                encoder_jb(jb)
            xpred(1, *og0[1])
            for jb in range(18, 22):
                encoder_jb(jb)
            og1 = rot_group([2, 3])
            for jb in range(22, 24):
                encoder_jb(jb)
            xpred(2, *og1[2])
            for jb in range(24, 26):
                encoder_jb(jb)
            xpred(3, *og1[3])
            for jb in range(26, 32):
                encoder_jb(jb)
            rp.release()
    nc.finalize()
    return nc


def _host_prep(inputs):
    f32, bf = np.float32, ml_dtypes.bfloat16
    xs = np.asarray(inputs["xs"], f32)
    perm = np.concatenate([np.arange(0, LDIM, 2), np.arange(1, LDIM, 2)])

    w1g = np.asarray(inputs["enc_W1"], f32) * np.asarray(inputs["enc_bn_gamma"], f32)
    w1e = np.ascontiguousarray(w1g.T)                       # [16, 256] f32
    b1h = (np.asarray(inputs["enc_b1"], f32)
           + np.asarray(inputs["enc_W1"], f32) @ np.asarray(inputs["enc_bn_beta"], f32))
    w2 = np.asarray(inputs["enc_W2"], f32)
    w2sb = np.empty((128, 4, 128), f32)
    for ki in range(2):
        for mo in range(2):
            w2sb[:, ki * 2 + mo, :] = w2[mo * 128:(mo + 1) * 128,
                                         ki * 128:(ki + 1) * 128].T
    b2 = np.asarray(inputs["enc_b2"], f32)
    b2c = np.ascontiguousarray(b2.reshape(2, 128).T)        # [128, 2]
    w3p = (np.asarray(inputs["enc_W3"], f32)
           * np.asarray(inputs["enc_scale"], f32)[:, None])[perm]   # [64, 256]
    w3sb = np.empty((128, 2, LDIM), f32)
    for ki in range(2):
        w3sb[:, ki, :] = w3p[:, ki * 128:(ki + 1) * 128].T

    w1a = (np.asarray(inputs["aux_W1"], f32)
           * np.asarray(inputs["aux_bn_gamma"], f32)[:, None, :])
    w1asb = np.ascontiguousarray(w1a.reshape(NAUX * AH, DIM).T)     # [16, 4096]
    b1a = (np.asarray(inputs["aux_b1"], f32)
           + np.einsum("kji,ki->kj", np.asarray(inputs["aux_W1"], f32),
                       np.asarray(inputs["aux_bn_beta"], f32)))
    b2a = np.asarray(inputs["aux_b2"], f32)
    w2asb = np.ascontiguousarray(
        np.asarray(inputs["aux_W2"], f32).transpose(2, 0, 1))       # [128, 32, 128]
    b2ac = np.ascontiguousarray(b2a.T)                              # [128, 32]
    w3adt = (np.asarray(inputs["aux_W3"], f32)
             * np.asarray(inputs["aux_scale"], f32)[:, :, None] * DT)
    w3asb = np.zeros((AH, NAUX, LDIM), f32)
    for k in range(NAUX):
        w3asb[:, k, k] = w3adt[k, 0, :]          # mu -> row k
        w3asb[:, k, 32 + k] = w3adt[k, 1, :]     # om -> row 32+k
    cw = np.asarray(inputs["Cw"], f32)                      # [16, 64]
    cwE, cwO = cw[:, 0::2], cw[:, 1::2]                     # [16, 32]
    cwe = np.zeros((128, 64), f32)
    cwo = np.zeros((128, 64), f32)
    for tsub in range(4):
        cwe[tsub * 32:(tsub + 1) * 32, tsub * 16:(tsub + 1) * 16] = cwE.T
        cwo[tsub * 32:(tsub + 1) * 32, tsub * 16:(tsub + 1) * 16] = cwO.T
    tvk = np.broadcast_to(np.arange(T, dtype=f32)[None, :, None],
                          (128, T, 32))

    shared = dict(
        w1e=w1e, b1r=np.ascontiguousarray(b1h.reshape(1, H)),
        w2=w2sb.astype(bf), b2c=b2c, w3=w3sb.astype(bf),
        w1a=w1asb, b1ar=b1a.reshape(1, NAUX * AH).astype(bf),
        w2a=w2asb.astype(bf), b2ac=b2ac, w3a=w3asb.astype(bf),
        cwe=cwe.astype(bf), cwo=cwo.astype(bf),
        tvk=np.ascontiguousarray(tvk).astype(bf),
        id16=np.eye(DIM, dtype=f32),
        onesr=np.ones((1, BC), f32).astype(bf))
    zb = bool(np.all(b2 == 0.0) and np.all(b2a == 0.0))
    in_maps = []
    for c in range(NCORES):
        xc = xs[c * BC:(c + 1) * BC]                        # [512, 64, 16]
        xsT = np.empty((D1, T, BC), f32)
        xsT[0:DIM] = xc.transpose(2, 1, 0)
        xsT[DIM] = 1.0
        xsN = np.concatenate(
            [xc.reshape(COLS, DIM), np.ones((COLS, 1), f32),
             np.zeros((COLS, 1), f32)], axis=1)             # [32768, 18]
        # partition-major packing: row (g*128 + p) -> xsNg[p, g, :]
        xsNg = np.ascontiguousarray(
            xsN.reshape(256, 128, 18).transpose(1, 0, 2))
        x0r = xsN.reshape(BC, T, 18)[:, 0, :]               # [512, 18]
        xsN0 = np.ascontiguousarray(x0r.reshape(4, 128, 18).transpose(1, 0, 2))
        m = dict(shared)
        m["xsT"] = xsT.astype(bf)
        m["xsNg"] = xsNg.astype(bf)
        m["xsN0"] = xsN0.astype(bf)
        in_maps.append(m)
    return in_maps, zb


def _assemble(inputs, results):
    f32 = np.float32
    xs = np.asarray(inputs["xs"], f32)
    perm = np.concatenate([np.arange(0, LDIM, 2), np.arange(1, LDIM, 2)])
    y = np.empty((B, T, DIM + LDIM), f32)
    y_pred = np.empty((B, T, DIM + LDIM), f32)
    y[:, :, :DIM] = xs
    for c in range(NCORES):
        r = results[c]
        sl = slice(c * BC, (c + 1) * BC)
        ye = np.asarray(r["yencP"], f32).reshape(32, 2, LDIM, BC)
        yenc_dev = ye.transpose(3, 0, 1, 2).reshape(BC, T, LDIM)
        y[sl, :, 16 + perm] = yenc_dev
        yl0 = np.asarray(r["yl0"], f32).reshape(BC, T, 32)
        yl1 = np.asarray(r["yl1"], f32).reshape(BC, T, 32)
        ylf = np.empty((BC, T, LDIM), f32)
        ylf[:, :, 0::2] = yl0
        ylf[:, :, 1::2] = yl1
        y_pred[sl, :, DIM:] = ylf
        # xp [4, 2, 128, 512] -> [cc, h, (qq, tsub, i), (jt, bsub)]
        xpr = np.asarray(r["xp"], f32).reshape(4, 2, 2, 4, 16, 4, 128)
        xpr = xpr.transpose(0, 6, 1, 2, 5, 3, 4).reshape(BC, T, DIM)
        y_pred[sl, :, :DIM] = xpr
    y_pred[:, 0, :DIM] = xs[:, 0, :]
    return y, y_pred


_NC_CACHE = {}


def kernel(**inputs):
    in_maps, zb = _host_prep(inputs)
    key = ("nc", zb)
    if key not in _NC_CACHE:
        _NC_CACHE[key] = build(zb=zb)
    nc = _NC_CACHE[key]
    res = bass_utils.run_bass_kernel_spmd(nc, in_maps,
                                          core_ids=list(range(NCORES)))
    return _assemble(inputs, res.results)


# revision 63
# speedup vs baseline: 1.0257x; 1.0231x over previous
"""Trainium2 Bass kernel for nn_DENIS_JBF (Koopman Jordan-block forecast).

v3 strategy (pure data parallel, 8 cores, Bc = 512 per core), all-bf16:
  - BN stats: host-packed partition-major Gram input (one DMA), bf16 PE
    Gram matmuls, AllGather (cheaper than AllReduce) + 3-step local
    reduce; BN folded into L1 weights on device.
  - One-pass leaky-relu evictions: ACT Prelu(alpha) from PSUM with L2
    bias fused (or skipped entirely when the biases are all zero, per
    input_specs); a tunable subset of tiles goes to DVE (copy+STT) to
    balance engines.
  - All-bf16: PE 1 col/cycle, DVE 2x_1p fast mode, half-size outputs.
  - Rotation in closed form, [128b, t, k] tiles with in-op broadcasts;
    Sin/Exp batched to avoid ACT table thrash; ec/es on GPSIMD (Pool).
  - yenc dims permuted (even/odd split) via host-permuted enc_W3 so
    mu/om and y0 pairs are contiguous after one blocked DMA-transpose.
  - x_pred: o0/o1 blocked-DMA-transposed, block-diagonal Cw matmuls.
  - DMA queues by role: SP = loads (deep xst prefetch) + stats + xp;
    GPSIMD/SWDGE = bulk stores (yencP, yl); ACT = dma transposes.
"""

import os
import sys

import numpy as np
import ml_dtypes

for _p in ("/opt/trn_rl_repo", "/root/.axon_site/_ro/trn_rl_repo"):
    if os.path.isdir(_p) and _p not in sys.path:
        sys.path.insert(0, _p)

import concourse.bass as bass
from concourse import bacc
import concourse.mybir as mybir
import concourse.tile as tile
from concourse import bass_utils
import concourse.dve_ops as dve_ops
from concourse.dve_ops import DveOp
from concourse.dve_spec import Spec, Src0, C0, maxx, lower
from concourse.dve_uop import DveOpSpec


def _register_lrelu():
    """Single-pass leaky-relu as a custom DVE op: max(Src0*C0, Src0)."""
    if "LRELU_ANT" in dve_ops._SUB_OPCODE_FOR_NAME:
        return next(op for op in dve_ops.OPS if op.name == "LRELU_ANT")
    spec = Spec(
        body=maxx(Src0 * C0, Src0),
        reference=lambda in0, in1, s0, s1, imm2: np.maximum(
            in0.astype(np.float32) * s0, in0.astype(np.float32)),
    )
    row = dve_ops._CUSTOM_DVE_ROW_BASE + len(dve_ops.OPS)
    assert row < 0x20
    dve_ops._SUB_OPCODE_FOR_NAME["LRELU_ANT"] = row
    shas = {}
    for ver in ("v3", "v4"):
        uops = lower(spec, ver=ver)
        shas[ver] = DveOpSpec(name="LRELU_ANT", opcode=row, uops=uops,
                              rd1_en=False).sha(ver)
    op = DveOp("LRELU_ANT", spec, subdim=False, uops_sha=shas)
    dve_ops.OPS.append(op)
    dve_ops.CUSTOM_DVE_SPECS["LRELU_ANT"] = spec
    return op


LRELU_OP = _register_lrelu()

F32 = mybir.dt.float32
BF16 = mybir.dt.bfloat16
AF = mybir.ActivationFunctionType
OP = mybir.AluOpType
AX = mybir.AxisListType

NCORES = 8
B, T, DIM, LDIM, NAUX = 4096, 64, 16, 64, 32
H, AH = 256, 128
DT = 0.01
EPS = 1e-5
BC = B // NCORES            # 512
COLS = BC * T               # 32768, col = t*BC + b
D1 = DIM + 1                # 17: +ones row (L1 bias in matmul)
PI = float(np.pi)
PF = 8                      # xst prefetch depth

# --- engine-assignment tunables -----------------------------------------
ENC_L1_DVE = frozenset(j for j in range(64) if j % 2 == 1)
ENC_L2_DVE = frozenset(j for j in range(64) if j % 4 == 1)
AUX_L1_DVE = frozenset(range(16))


def build(zb=True):
    """zb: biases (enc_b2 / aux_b2) are all-zero -> single-op L2 evictions."""
    nc = bacc.Bacc(None)

    # ---------------- DRAM I/O ----------------
    xsT_d = nc.dram_tensor("xsT", [D1, T, BC], BF16, kind="ExternalInput")
    xsNg_d = nc.dram_tensor("xsNg", [128, 256, 18], BF16, kind="ExternalInput")
    xsN0_d = nc.dram_tensor("xsN0", [128, 4, 18], BF16, kind="ExternalInput")
    w1e_d = nc.dram_tensor("w1e", [DIM, H], F32, kind="ExternalInput")
    b1r_d = nc.dram_tensor("b1r", [1, H], F32, kind="ExternalInput")
    w2_d = nc.dram_tensor("w2", [128, 4, 128], BF16, kind="ExternalInput")
    b2c_d = nc.dram_tensor("b2c", [128, 2], F32, kind="ExternalInput")
    w3_d = nc.dram_tensor("w3", [128, 2, LDIM], BF16, kind="ExternalInput")
    w1a_d = nc.dram_tensor("w1a", [DIM, NAUX * AH], F32, kind="ExternalInput")
    b1ar_d = nc.dram_tensor("b1ar", [1, NAUX * AH], BF16, kind="ExternalInput")
    w2a_d = nc.dram_tensor("w2a", [AH, NAUX, AH], BF16, kind="ExternalInput")
    b2ac_d = nc.dram_tensor("b2ac", [128, NAUX], F32, kind="ExternalInput")
    w3a_d = nc.dram_tensor("w3a", [AH, NAUX, LDIM], BF16, kind="ExternalInput")
    cwe_d = nc.dram_tensor("cwe", [128, 64], BF16, kind="ExternalInput")
    cwo_d = nc.dram_tensor("cwo", [128, 64], BF16, kind="ExternalInput")
    tvk_d = nc.dram_tensor("tvk", [128, T, 32], BF16, kind="ExternalInput")
    id16_d = nc.dram_tensor("id16", [DIM, DIM], F32, kind="ExternalInput")
    onesr_d = nc.dram_tensor("onesr", [1, BC], BF16, kind="ExternalInput")

    yencP_o = nc.dram_tensor("yencP", [32, 128, BC], BF16, kind="ExternalOutput")
    yl0_o = nc.dram_tensor("yl0", [4, 128, T, 32], BF16, kind="ExternalOutput")
    yl1_o = nc.dram_tensor("yl1", [4, 128, T, 32], BF16, kind="ExternalOutput")
    xp_o = nc.dram_tensor("xp", [4, 2, 128, 512], BF16, kind="ExternalOutput")

    stat_in = nc.dram_tensor("stat_in", [2, 18, 18], F32)
    stat_out = nc.dram_tensor("stat_out", [NCORES, 2, 18, 18], F32,
                              addr_space="Shared")

    with tile.TileContext(nc) as tc:
        with tc.tile_pool(name="consts", bufs=1) as cp, \
             tc.tile_pool(name="psum", bufs=1, space="PSUM") as pp, \
             tc.tile_pool(name="stream", bufs=2) as sp, \
             tc.tile_pool(name="smalls", bufs=1) as smp:
            bp = tc.alloc_tile_pool(name="boot", bufs=1)

            # ------------- weights / consts (SP loads, overlap stats) ----
            def ld(shape, dt, dram, name):
                t = cp.tile(shape, dt, tag=name)
                nc.sync.dma_start(out=t, in_=dram)
                return t

            # stats inputs first: the Gram + AllGather is the serial prologue
            xg = bp.tile([128, 256, 18], BF16, tag="xg", bufs=1)
            nc.sync.dma_start(out=xg, in_=xsNg_d[:, :, :])
            x0g = bp.tile([128, 4, 18], BF16, tag="x0g", bufs=1)
            nc.sync.dma_start(out=x0g, in_=xsN0_d[:, :, :])
            w1e_sb = ld([DIM, H], F32, w1e_d[:, :], "w1e")
            b1r_sb = ld([1, H], F32, b1r_d[:, :], "b1r")
            w2_sb = ld([128, 4, 128], BF16, w2_d[:, :, :], "w2")
            b2c_sb = ld([128, 2], F32, b2c_d[:, :], "b2c")
            w3_sb = ld([128, 2, LDIM], BF16, w3_d[:, :, :], "w3")
            w2a_sb = ld([AH, NAUX, AH], BF16, w2a_d[:, :, :], "w2a")
            b2ac_sb = ld([128, NAUX], F32, b2ac_d[:, :], "b2ac")
            w3a_sb = ld([AH, NAUX, LDIM], BF16, w3a_d[:, :, :], "w3a")
            cwe_sb = ld([128, 64], BF16, cwe_d[:, :], "cwe")
            cwo_sb = ld([128, 64], BF16, cwo_d[:, :], "cwo")
            tvk_sb = ld([128, T, 32], BF16, tvk_d[:, :, :], "tvk")
            id16_sb = ld([DIM, DIM], F32, id16_d[:, :], "id16")
            w1a_sb = bp.tile([DIM, NAUX * AH], F32, tag="w1atmp", bufs=1)
            nc.sync.dma_start(out=w1a_sb, in_=w1a_d[:, :])
            x0T = bp.tile([DIM, BC], BF16, tag="x0T", bufs=1)
            nc.sync.dma_start(out=x0T, in_=xsT_d[0:DIM, 0, :])

            hpib = cp.tile([128, 1], F32)
            nc.vector.memset(hpib, PI / 2.0)
            epsb = cp.tile([128, 1], F32)
            nc.vector.memset(epsb, EPS)

            # xst stream with deep prefetch (pure loads on SP queue)
            def load_xst(jb):
                xst = sp.tile([D1, 2, 512], BF16, tag="xst", bufs=PF)
                nc.sync.dma_start(
                    out=xst, in_=xsT_d[:, jb * 2:(jb + 1) * 2, :])
                return xst

            xst_tiles = {}
            for jb in range(min(PF, 32)):
                xst_tiles[jb] = load_xst(jb)

            # ------------- phase A: Gram stats + AllGather -------------
            pg0 = pp.tile([18, 18], F32, tag="pC", bufs=2)
            for g in range(4):
                nc.tensor.matmul(pg0[:, :], x0g[:, g, :], x0g[:, g, :],
                                 start=(g == 0), stop=(g == 3))
            pg = pp.tile([18, 18], F32, tag="pA0", bufs=2)
            for g in range(256):
                nc.tensor.matmul(pg[:, :], xg[:, g, :], xg[:, g, :],
                                 start=(g == 0), stop=(g == 255))
            gA = smp.tile([18, 18], F32)
            nc.vector.tensor_copy(gA, pg[:, :])
            gB = smp.tile([18, 18], F32)
            nc.vector.tensor_copy(gB, pg0[:, :])
            nc.gpsimd.dma_start(out=stat_in[0, :, :], in_=gA[:])
            nc.gpsimd.dma_start(out=stat_in[1, :, :], in_=gB[:])
            nc.gpsimd.collective_compute(
                "AllGather", OP.bypass, replica_groups=[list(range(NCORES))],
                ins=[stat_in[:, :, :]], outs=[stat_out[:, :, :, :]])
            g8 = smp.tile([18, NCORES, 2, 18], F32)
            nc.gpsimd.dma_start(out=g8,
                                in_=stat_out[:, :, :, :].transpose([2, 0, 1, 3]))
            r4 = smp.tile([18, 4, 2, 18], F32)
            nc.vector.tensor_add(r4, g8[:, 0:4, :, :], g8[:, 4:8, :, :])
            r2 = smp.tile([18, 2, 2, 18], F32)
            nc.vector.tensor_add(r2, r4[:, 0:2, :, :], r4[:, 2:4, :, :])
            stats = smp.tile([18, 2, 18], F32)
            nc.vector.tensor_add(stats, r2[:, 0, :, :], r2[:, 1, :, :])

            # ------------- phase F: fold BN into L1 weights -------------
            def fold(set_idx, n):
                g16 = stats[0:DIM, set_idx, 0:DIM]
                scol = stats[0:DIM, set_idx, DIM:DIM + 1]
                m = smp.tile([DIM, 1], F32, tag=f"m{set_idx}")
                nc.vector.tensor_scalar(m, scol, 1.0 / n, None, OP.mult)
                gi = smp.tile([DIM, DIM], F32, tag=f"gi{set_idx}")
                nc.vector.tensor_mul(gi, g16, id16_sb[:])
                qd = smp.tile([DIM, 1], F32, tag=f"qd{set_idx}")
                nc.vector.reduce_sum(qd, gi, axis=AX.X)
                m2 = smp.tile([DIM, 1], F32, tag=f"m2{set_idx}")
                nc.vector.tensor_mul(m2, m, m)
                v = smp.tile([DIM, 1], F32, tag=f"v{set_idx}")
                nc.vector.scalar_tensor_tensor(v, qd, 1.0 / n, m2,
                                               OP.mult, OP.subtract)
                # rs = (v+eps)^-1/2 via Ln+Exp (one shared act table set)
                lnv = smp.tile([DIM, 1], F32, tag=f"ln{set_idx}")
                nc.scalar.activation(lnv, v, AF.Ln, bias=epsb[0:DIM, :])
                rs = smp.tile([DIM, 1], F32, tag=f"rs{set_idx}")
                nc.scalar.activation(rs, lnv, AF.Exp, scale=-0.5)
                return m, rs

            m_all, rs_all = fold(0, float(B * T))
            m_0, rs_0 = fold(1, float(B))

            w1es = cp.tile([D1, H], BF16)
            nc.vector.tensor_scalar(w1es[0:DIM, :], w1e_sb[:], rs_all[:],
                                    None, OP.mult)
            mneg = smp.tile([DIM, 1], BF16)
            nc.vector.tensor_scalar(mneg, m_all[:], -1.0, None, OP.mult)
            pbc = pp.tile([1, H], F32, tag="pC", bufs=2)
            nc.tensor.matmul(pbc[:, :], mneg[:], w1es[0:DIM, :],
                             start=True, stop=True)
            badd = smp.tile([1, H], BF16)
            nc.vector.tensor_add(badd, pbc[:, :], b1r_sb[:])
            nc.sync.dma_start(out=w1es[DIM:D1, :], in_=badd[:])

            w1as = cp.tile([D1, NAUX * AH], BF16)
            nc.vector.tensor_scalar(w1as[0:DIM, 0:2048], w1a_sb[:, 0:2048],
                                    rs_0[:], None, OP.mult)
            nc.scalar.activation(w1as[0:DIM, 2048:4096],
                                 w1a_sb[:, 2048:4096], AF.Copy,
                                 scale=rs_0[:])
            nc.sync.dma_start(out=w1as[DIM:D1, :], in_=b1ar_d[:, :])

            m0n = smp.tile([DIM, 1], F32)
            nc.vector.tensor_scalar(m0n, m_0[:], -1.0, None, OP.mult)
            x0c = cp.tile([D1, BC], BF16)
            nc.vector.tensor_scalar(x0c[0:DIM, :], x0T[:], m0n[:],
                                    None, OP.add)
            nc.sync.dma_start(out=x0c[DIM:D1, :], in_=onesr_d[:, :])
            bp.release()
            rp = tc.alloc_tile_pool(name="rot", bufs=1)

            # ------------- phase X: aux nets -> pmw [64, 512] -------------
            pmw = pp.tile([LDIM, BC], F32, tag="pC", bufs=2)
            for kp in range(NAUX // 2):
                h1a = sp.tile([128, 2, BC], BF16, tag="h1a", bufs=2)
                pa1 = [pp.tile([128, BC], F32, tag=f"pA{s}", bufs=2,
                               name=f"pa1_{s}") for s in range(2)]
                for s in range(2):
                    k = kp * 2 + s
                    nc.tensor.matmul(pa1[s][:, :],
                                     w1as[:, k * AH:(k + 1) * AH],
                                     x0c[:], start=True, stop=True)
                for s in range(2):
                    if kp in AUX_L1_DVE:
                        nc.vector._custom_dve(LRELU_OP, out=h1a[:, s, :],
                                              in0=pa1[s][:, :], s0=0.01)
                    else:
                        nc.scalar.activation(h1a[:, s, :], pa1[s][:, :],
                                             AF.Prelu, alpha=0.01)
                h2a = sp.tile([128, 2, BC], BF16, tag="h2a", bufs=2)
                for s in range(2):
                    k = kp * 2 + s
                    pa2 = pp.tile([128, BC], F32, tag=f"pB{s}", bufs=1)
                    nc.tensor.matmul(pa2[:, :], w2a_sb[:, k, :],
                                     h1a[:, s, :], start=True, stop=True)
                    if zb:
                        nc.scalar.activation(h2a[:, s, :], pa2[:, :],
                                             AF.Prelu, alpha=0.01)
                    else:
                        nc.scalar.activation(h2a[:, s, :], pa2[:, :],
                                             AF.Prelu,
                                             bias=b2ac_sb[:, k:k + 1],
                                             alpha=0.01)
                    nc.tensor.matmul(pmw[:, :], w3a_sb[:, k, :],
                                     h2a[:, s, :],
                                     start=(k == 0), stop=(k == NAUX - 1))

            # muom -> SBUF bf16, blocked transpose to b-major
            muomS = cp.tile([LDIM, BC], BF16)
            nc.vector.tensor_copy(muomS, pmw[:, :])
            mT = cp.tile([128, 4, LDIM], BF16)
            nc.sync.dma_start_transpose(mT, muomS[:])
            # mT[:, c, 0:32] = mu, mT[:, c, 32:64] = om

            # ------------- phase E: encoder -------------
            y0S = cp.tile([LDIM, BC], BF16)
            yT = cp.tile([128, 4, LDIM], BF16)

            def encoder_jb(jb):
                if jb + PF < 32:
                    xst_tiles[jb + PF] = load_xst(jb + PF)
                xst = xst_tiles.pop(jb)
                p3 = pp.tile([128, 512], F32, tag="pC", bufs=2)
                for jj in range(2):
                    j = jb * 2 + jj
                    rhs = xst[:, jj, :]
                    h1 = sp.tile([128, 2, 512], BF16, tag="h1")
                    p1 = [pp.tile([128, 512], F32, tag=f"pA{mo}", bufs=2,
                                  name=f"p1_{mo}") for mo in range(2)]
                    for mo in range(2):
                        nc.tensor.matmul(p1[mo][:, :],
                                         w1es[:, mo * 128:(mo + 1) * 128],
                                         rhs, start=True, stop=True)
                    for mo in range(2):
                        if j in ENC_L1_DVE:
                            nc.vector._custom_dve(LRELU_OP, out=h1[:, mo, :],
                                                  in0=p1[mo][:, :], s0=0.01)
                        else:
                            nc.scalar.activation(h1[:, mo, :], p1[mo][:, :],
                                                 AF.Prelu, alpha=0.01)
                    h2 = sp.tile([128, 2, 512], BF16, tag="h2")
                    for mo in range(2):
                        p2 = pp.tile([128, 512], F32, tag=f"pB{mo}", bufs=1)
                        for ki in range(2):
                            nc.tensor.matmul(p2[:, :],
                                             w2_sb[:, ki * 2 + mo, :],
                                             h1[:, ki, :],
                                             start=(ki == 0), stop=(ki == 1))
                        if zb and j in ENC_L2_DVE:
                            nc.vector._custom_dve(LRELU_OP, out=h2[:, mo, :],
                                                  in0=p2[:, :], s0=0.01)
                        elif zb:
                            nc.scalar.activation(h2[:, mo, :], p2[:, :],
                                                 AF.Prelu, alpha=0.01)
                        else:
                            nc.scalar.activation(h2[:, mo, :], p2[:, :],
                                                 AF.Prelu,
                                                 bias=b2c_sb[:, mo:mo + 1],
                                                 alpha=0.01)
                        nc.tensor.matmul(p3[jj * 64:jj * 64 + 64, :],
                                         w3_sb[:, mo, :], h2[:, mo, :],
                                         start=(mo == 0), stop=(mo == 1))
                yt = sp.tile([128, 512], BF16, tag="yt")
                nc.vector.tensor_copy(yt[:], p3[:, :])
                nc.gpsimd.dma_start(out=yencP_o[jb, :, :], in_=yt[:])
                return yt

            yt0 = encoder_jb(0)
            nc.vector.tensor_copy(y0S, yt0[0:LDIM, :])
            nc.sync.dma_start_transpose(yT, y0S[:])
            # yT[:, c, 0:32] = y00, yT[:, c, 32:64] = y01

            # ------------- rotation (2 groups, interleaved) -------------
            angs, rads, ees = {}, {}, {}

            def rot_angles(cs):
                for c in cs:
                    om = mT[:, c, 32:64].unsqueeze(1) \
                        .broadcast_to([128, T, 32])
                    mu = mT[:, c, 0:32].unsqueeze(1) \
                        .broadcast_to([128, T, 32])
                    ang = rp.tile([128, T, 32], BF16, tag="ang", bufs=4)
                    nc.vector.tensor_mul(ang, tvk_sb[:], om)
                    rad = rp.tile([128, T, 32], BF16, tag="rad", bufs=4)
                    nc.vector.tensor_mul(rad, tvk_sb[:], mu)
                    angs[c], rads[c] = ang, rad
                for c in cs:       # one Exp table load for ALL chunks
                    ee = rp.tile([128, T, 32], BF16, tag="ee", bufs=4)
                    nc.scalar.activation(ee, rads[c][:], AF.Exp)
                    ees[c] = ee

            def rot_group(cs):
                sss, ccs = {}, {}
                outs = {}
                for c in cs:       # batched: one Sin table load
                    ss = rp.tile([128, T, 32], BF16, tag="ss", bufs=2)
                    nc.scalar.activation(ss, angs[c][:], AF.Sin)
                    cc = rp.tile([128, T, 32], BF16, tag="cc", bufs=2)
                    nc.scalar.activation(cc, angs[c][:], AF.Sin,
                                         bias=hpib[:, :], scale=-1.0)
                    sss[c], ccs[c] = ss, cc
                for c in cs:
                    y00 = yT[:, c, 0:32].unsqueeze(1) \
                        .broadcast_to([128, T, 32])
                    y01 = yT[:, c, 32:64].unsqueeze(1) \
                        .broadcast_to([128, T, 32])
                    ec = rp.tile([128, T, 32], BF16, tag="ec", bufs=2)
                    nc.gpsimd.tensor_mul(ec, ees[c][:], ccs[c][:])
                    es = rp.tile([128, T, 32], BF16, tag="es", bufs=2)
                    nc.vector.tensor_mul(es, ees[c][:], sss[c][:])
                    t2 = rp.tile([128, T, 32], BF16, tag="t2", bufs=1)
                    nc.vector.tensor_mul(t2, es[:], y01)
                    t3 = rp.tile([128, T, 32], BF16, tag="t3", bufs=1)
                    nc.vector.tensor_mul(t3, es[:], y00)
                    t1 = rp.tile([128, T, 32], BF16, tag="t1", bufs=1)
                    nc.vector.tensor_mul(t1, ec[:], y00)
                    o0 = rp.tile([128, T, 32], BF16, tag="o0", bufs=2)
                    nc.vector.tensor_sub(o0, t1[:], t2[:])
                    nc.gpsimd.dma_start(out=yl0_o[c, :, :, :], in_=o0[:])
                    t4 = rp.tile([128, T, 32], BF16, tag="t4", bufs=1)
                    nc.vector.tensor_mul(t4, ec[:], y01)
                    o1 = rp.tile([128, T, 32], BF16, tag="o1", bufs=2)
                    nc.vector.tensor_add(o1, t3[:], t4[:])
                    nc.gpsimd.dma_start(out=yl1_o[c, :, :, :], in_=o1[:])
                    xin0 = rp.tile([128, 16, 128], BF16, tag="xin0", bufs=2)
                    nc.sync.dma_start_transpose(
                        xin0, o0[:].rearrange("p t k -> p (t k)"))
                    xin1 = rp.tile([128, 16, 128], BF16, tag="xin1", bufs=2)
                    nc.sync.dma_start_transpose(
                        xin1, o1[:].rearrange("p t k -> p (t k)"))
                    outs[c] = (xin0, xin1)
                return outs

            def xpred(c, xin0, xin1):
                for h in range(2):
                    pxp = pp.tile([128, 512], F32, tag="pC", bufs=2)
                    for qq in range(2):
                        q = h * 2 + qq
                        sl = slice(64 * qq, 64 * qq + 64)
                        nc.tensor.matmul(pxp[sl, :], cwe_sb[:],
                                         xin0[:, 4 * q:4 * q + 4, :],
                                         start=True, stop=False)
                        nc.tensor.matmul(pxp[sl, :], cwo_sb[:],
                                         xin1[:, 4 * q:4 * q + 4, :],
                                         start=False, stop=True)
                    xpt = sp.tile([128, 512], BF16, tag="xpt")
                    nc.vector.tensor_copy(xpt[:], pxp[:, :])
                    nc.sync.dma_start(out=xp_o[c, h, :, :], in_=xpt[:])

            rot_angles([0, 1, 2, 3])
            for jb in range(1, 14):
                encoder_jb(jb)
            og0 = rot_group([0, 1])
            for jb in range(14, 16):
                encoder_jb(jb)
            xpred(0, *og0[0])
            for jb in range(16, 18):
                encoder_jb(jb)
            xpred(1, *og0[1])
            for jb in range(18, 22):
                encoder_jb(jb)
            og1 = rot_group([2, 3])
            for jb in range(22, 24):
                encoder_jb(jb)
            xpred(2, *og1[2])
            for jb in range(24, 26):
                encoder_jb(jb)
            xpred(3, *og1[3])
            for jb in range(26, 32):
                encoder_jb(jb)
            rp.release()
    nc.finalize()
    return nc


def _host_prep(inputs):
    f32, bf = np.float32, ml_dtypes.bfloat16
    xs = np.asarray(inputs["xs"], f32)
    perm = np.concatenate([np.arange(0, LDIM, 2), np.arange(1, LDIM, 2)])

    w1g = np.asarray(inputs["enc_W1"], f32) * np.asarray(inputs["enc_bn_gamma"], f32)
    w1e = np.ascontiguousarray(w1g.T)                       # [16, 256] f32
    b1h = (np.asarray(inputs["enc_b1"], f32)
           + np.asarray(inputs["enc_W1"], f32) @ np.asarray(inputs["enc_bn_beta"], f32))
    w2 = np.asarray(inputs["enc_W2"], f32)
    w2sb = np.empty((128, 4, 128), f32)
    for ki in range(2):
        for mo in range(2):
            w2sb[:, ki * 2 + mo, :] = w2[mo * 128:(mo + 1) * 128,
                                         ki * 128:(ki + 1) * 128].T
    b2 = np.asarray(inputs["enc_b2"], f32)
    b2c = np.ascontiguousarray(b2.reshape(2, 128).T)        # [128, 2]
    w3p = (np.asarray(inputs["enc_W3"], f32)
           * np.asarray(inputs["enc_scale"], f32)[:, None])[perm]   # [64, 256]
    w3sb = np.empty((128, 2, LDIM), f32)
    for ki in range(2):
        w3sb[:, ki, :] = w3p[:, ki * 128:(ki + 1) * 128].T

    w1a = (np.asarray(inputs["aux_W1"], f32)
           * np.asarray(inputs["aux_bn_gamma"], f32)[:, None, :])
    w1asb = np.ascontiguousarray(w1a.reshape(NAUX * AH, DIM).T)     # [16, 4096]
    b1a = (np.asarray(inputs["aux_b1"], f32)
           + np.einsum("kji,ki->kj", np.asarray(inputs["aux_W1"], f32),
                       np.asarray(inputs["aux_bn_beta"], f32)))
    b2a = np.asarray(inputs["aux_b2"], f32)
    w2asb = np.ascontiguousarray(
        np.asarray(inputs["aux_W2"], f32).transpose(2, 0, 1))       # [128, 32, 128]
    b2ac = np.ascontiguousarray(b2a.T)                              # [128, 32]
    w3adt = (np.asarray(inputs["aux_W3"], f32)
             * np.asarray(inputs["aux_scale"], f32)[:, :, None] * DT)
    w3asb = np.zeros((AH, NAUX, LDIM), f32)
    for k in range(NAUX):
        w3asb[:, k, k] = w3adt[k, 0, :]          # mu -> row k
        w3asb[:, k, 32 + k] = w3adt[k, 1, :]     # om -> row 32+k
    cw = np.asarray(inputs["Cw"], f32)                      # [16, 64]
    cwE, cwO = cw[:, 0::2], cw[:, 1::2]                     # [16, 32]
    cwe = np.zeros((128, 64), f32)
    cwo = np.zeros((128, 64), f32)
    for tsub in range(4):
        cwe[tsub * 32:(tsub + 1) * 32, tsub * 16:(tsub + 1) * 16] = cwE.T
        cwo[tsub * 32:(tsub + 1) * 32, tsub * 16:(tsub + 1) * 16] = cwO.T
    tvk = np.broadcast_to(np.arange(T, dtype=f32)[None, :, None],
                          (128, T, 32))

    shared = dict(
        w1e=w1e, b1r=np.ascontiguousarray(b1h.reshape(1, H)),
        w2=w2sb.astype(bf), b2c=b2c, w3=w3sb.astype(bf),
        w1a=w1asb, b1ar=b1a.reshape(1, NAUX * AH).astype(bf),
        w2a=w2asb.astype(bf), b2ac=b2ac, w3a=w3asb.astype(bf),
        cwe=cwe.astype(bf), cwo=cwo.astype(bf),
        tvk=np.ascontiguousarray(tvk).astype(bf),
        id16=np.eye(DIM, dtype=f32),
        onesr=np.ones((1, BC), f32).astype(bf))
    zb = bool(np.all(b2 == 0.0) and np.all(b2a == 0.0))
    in_maps = []
    for c in range(NCORES):
        xc = xs[c * BC:(c + 1) * BC]                        # [512, 64, 16]
        xsT = np.empty((D1, T, BC), f32)
        xsT[0:DIM] = xc.transpose(2, 1, 0)
        xsT[DIM] = 1.0
        xsN = np.concatenate(
            [xc.reshape(COLS, DIM), np.ones((COLS, 1), f32),
             np.zeros((COLS, 1), f32)], axis=1)             # [32768, 18]
        # partition-major packing: row (g*128 + p) -> xsNg[p, g, :]
        xsNg = np.ascontiguousarray(
            xsN.reshape(256, 128, 18).transpose(1, 0, 2))
        x0r = xsN.reshape(BC, T, 18)[:, 0, :]               # [512, 18]
        xsN0 = np.ascontiguousarray(x0r.reshape(4, 128, 18).transpose(1, 0, 2))
        m = dict(shared)
        m["xsT"] = xsT.astype(bf)
        m["xsNg"] = xsNg.astype(bf)
        m["xsN0"] = xsN0.astype(bf)
        in_maps.append(m)
    return in_maps, zb


def _assemble(inputs, results):
    f32 = np.float32
    xs = np.asarray(inputs["xs"], f32)
    perm = np.concatenate([np.arange(0, LDIM, 2), np.arange(1, LDIM, 2)])
    y = np.empty((B, T, DIM + LDIM), f32)
    y_pred = np.empty((B, T, DIM + LDIM), f32)
    y[:, :, :DIM] = xs
    for c in range(NCORES):
        r = results[c]
        sl = slice(c * BC, (c + 1) * BC)
        ye = np.asarray(r["yencP"], f32).reshape(32, 2, LDIM, BC)
        yenc_dev = ye.transpose(3, 0, 1, 2).reshape(BC, T, LDIM)
        y[sl, :, 16 + perm] = yenc_dev
        yl0 = np.asarray(r["yl0"], f32).reshape(BC, T, 32)
        yl1 = np.asarray(r["yl1"], f32).reshape(BC, T, 32)
        ylf = np.empty((BC, T, LDIM), f32)
        ylf[:, :, 0::2] = yl0
        ylf[:, :, 1::2] = yl1
        y_pred[sl, :, DIM:] = ylf
        # xp [4, 2, 128, 512] -> [cc, h, (qq, tsub, i), (jt, bsub)]
        xpr = np.asarray(r["xp"], f32).reshape(4, 2, 2, 4, 16, 4, 128)
        xpr = xpr.transpose(0, 6, 1, 2, 5, 3, 4).reshape(BC, T, DIM)
        y_pred[sl, :, :DIM] = xpr
    y_pred[:, 0, :DIM] = xs[:, 0, :]
    return y, y_pred


_NC_CACHE = {}


def kernel(**inputs):
    in_maps, zb = _host_prep(inputs)
    key = ("nc", zb)
    if key not in _NC_CACHE:
        _NC_CACHE[key] = build(zb=zb)
    nc = _NC_CACHE[key]
    res = bass_utils.run_bass_kernel_spmd(nc, in_maps,
                                          core_ids=list(range(NCORES)))
    return _assemble(inputs, res.results)
